# revision 28
# baseline (speedup 1.0000x reference)
"""Multi-head attention forward, tensor-parallel over 8 TRN2 NeuronCores.

Problem: x[4,2048,1024], Wqkv[1024,3072], bqkv[3072], Wo[1024,1024], bo[1024]
  qkv = x @ Wqkv + bqkv ; 16 heads, d_head 64 ; softmax(QK^T/8) V ; out proj.

Sharding: DP=2 over batch (2 batches/core) x TP=4 over heads (4 heads/core).
Each core computes a partial y^T (its heads' contribution, transposed); the
host sums partials within each batch group, adds biases, and transposes.

Device dataflow (v2 — fully SBUF-resident, head-pair row tiling):
  qT,kT = (W_{q,k}^T x^T + b)   [128, 2048] per (batch, head-pair):
                                partitions 0-63 = even head, 64-127 = odd head
  v     = x W_v                 [128 tok, 256] strips -> vtmp -> vo per head
  S^T   = K Q^T per (b,hp,qb,strip): TWO heads concurrently as 64-row PE
          tiles T0/T8 (K=64, operands live on partition halves) -> one
          2-bank PSUM tile [128, 2, 512]
  P^T   = exp(S^T/8)            one ACT call, N=1024, bf16 out
  O^T|s = [V|1]^T P^T           per head, [65, 512] PSUM accumulated over 16
                                strips; row 64 = rowsum
  norm  : reciprocal_approx_fast on rowsums, f32r broadcast matmul (e65),
          DVE mul -> ot tiles [128, 2048] (odd head shifted via SBUF DMA)
  y^T  += Wo_part^T O_n^T       [1024, 2048] partial per batch, summed on host
"""

import sys

if "/opt/trn_rl_repo" not in sys.path:
    sys.path.insert(0, "/opt/trn_rl_repo")

import numpy as np

B, S, D = 4, 2048, 1024
H, DH = 16, 64
NCORES = 8
DP, TP = 2, 4
BL = B // DP            # 2 local batches
TOK = BL * S            # 4096 local tokens
HL = H // TP            # 4 local heads
HD = HL * DH            # 256 local head dims
P = 128
NF = D // P             # 8 contraction chunks
CH = 512                # token chunk for projections
NJ = S // CH            # 4 chunks per batch
NKS = S // P            # 16 k-strips per (batch, head)
QB = 512                # q block (one PSUM bank)
NQB = S // QB           # 4 q blocks per head

_cache = {}


def _build():
    import concourse.bass as bass
    import concourse.tile as tile
    from concourse import bacc, mybir
    from contextlib import ExitStack

    FP = mybir.dt.float32
    FR = mybir.dt.bfloat16
    F32R = mybir.dt.float32r
    AF = mybir.ActivationFunctionType

    nc = bacc.Bacc("TRN2", target_bir_lowering=False, debug=False,
                   num_devices=NCORES)

    xT = nc.dram_tensor("xT", [D, TOK], FR, kind="ExternalInput").ap()
    w = nc.dram_tensor("w", [D, 3 * HD], FR, kind="ExternalInput").ap()
    bqk = nc.dram_tensor("bqk", [2 * HD, 1], FP, kind="ExternalInput").ap()
    wo = nc.dram_tensor("wo", [HD, D], FR, kind="ExternalInput").ap()
    yT = nc.dram_tensor("yT", [D, TOK], FP, kind="ExternalOutput").ap()

    with tile.TileContext(nc) as tc, ExitStack() as ctx:
        konst = ctx.enter_context(tc.tile_pool(name="konst", bufs=1))
        xt_p = ctx.enter_context(tc.tile_pool(name="xt", bufs=1))
        qp_p = ctx.enter_context(tc.tile_pool(name="qp", bufs=1))
        kp_p = ctx.enter_context(tc.tile_pool(name="kp", bufs=1))
        vt_p = ctx.enter_context(tc.tile_pool(name="vt", bufs=1))
        vo_p = ctx.enter_context(tc.tile_pool(name="vo", bufs=1))
        ot_p = ctx.enter_context(tc.tile_pool(name="ot", bufs=1))
        pt_p = ctx.enter_context(tc.tile_pool(name="pt", bufs=12))
        stage = ctx.enter_context(tc.tile_pool(name="stage", bufs=6))
        s_ps = ctx.enter_context(
            tc.tile_pool(name="sps", bufs=2, space="PSUM"))
        pv_ps = ctx.enter_context(
            tc.tile_pool(name="pvps", bufs=1, space="PSUM"))
        mm_ps = ctx.enter_context(
            tc.tile_pool(name="mmps", bufs=2, space="PSUM"))

        # ---- constants resident in SBUF ----
        w_t = konst.tile([P, NF, 3 * HD], FR, tag="w")
        for f in range(NF):
            nc.sync.dma_start(w_t[:, f, :], w[f * P:(f + 1) * P, :])
        wo_t = konst.tile([P, 2, D], FR, tag="wo")
        for kc in range(2):
            nc.sync.dma_start(wo_t[:, kc, :], wo[kc * P:(kc + 1) * P, :])
        bias_t = konst.tile([P, 4], FP, tag="bias")
        for o in range(4):
            nc.sync.dma_start(bias_t[:, o:o + 1], bqk[o * P:(o + 1) * P, :])
        # e65: selects row 64 (the rowsum) in the broadcast matmul (f32r for
        # full PE rate; memset through an fp32 bitcast view)
        e65 = konst.tile([DH + 1, DH], FR, tag="e65")
        nc.gpsimd.memset(e65[:], 0.0)
        nc.gpsimd.memset(e65[DH:DH + 1, :], 1.0)
        # reciprocal staging: row 64 written per norm event; rows 0..63 are
        # constant (multiplied by e65's zeros) but must stay finite
        rcp_t = konst.tile([DH + 1, 2, QB], FR, tag="rcp")
        nc.gpsimd.memset(rcp_t[:], 1.0)
        rcps = konst.tile([DH + 1, 2, QB], FP, tag="rcps")
        rcpi = konst.tile([DH + 1, 2, QB], FP, tag="rcpi")
        nc.gpsimd.memset(rcpi[:], 1.0)
        # fp32 ones row used to fill the vo ones column
        ones16 = konst.tile([P, NKS], FP, tag="ones16")
        nc.gpsimd.memset(ones16[:], 1.0)
        # ACT exp table warm-up (first Exp pays ~2.7us table DMA)
        warm = konst.tile([1, 4], FR, tag="warm")
        nc.scalar.activation(warm[:], bias_t[0:1, :], AF.Exp, scale=0.125)

        # ---- persistent SBUF tensors ----
        # x for one batch (reused batch 1 over batch 0 via WAR deps)
        xt_t = xt_p.tile([P, NF, NJ, CH], FR, tag="xt", name="xt_t")

        # q/k: [128, 2048] per (batch, head-pair); partitions 0-63 even head
        qp = [[qp_p.tile([P, S], FR, tag=f"qp{b}_{hp}", name=f"qp{b}_{hp}")
               for hp in range(2)] for b in range(BL)]
        kp = [[kp_p.tile([P, S], FR, tag=f"kp{b}_{hp}", name=f"kp{b}_{hp}")
               for hp in range(2)] for b in range(BL)]
        # v staging [128 tok, strip, 256 feat] and per-head [V|1] tiles
        vtmp = [vt_p.tile([P, NKS, HD], FR, tag=f"vt{b}", name=f"vt{b}")
                for b in range(BL)]
        vo = [[vo_p.tile([P, NKS, DH + 1], FR, tag=f"vo{b}_{h}",
                         name=f"vo{b}_{h}") for h in range(HL)]
              for b in range(BL)]
        for b in range(BL):
            for h in range(HL):
                nc.vector.tensor_copy(vo[b][h][:, :, DH], ones16[:])
        # normalized O^T, 2 heads stacked per tile
        ot = [[ot_p.tile([P, S], FR, tag=f"ot{b}_{hp}", name=f"ot{b}_{hp}")
               for hp in range(2)] for b in range(BL)]

        def ldx(b, j=None):
            """Queue x DMAs for batch b: chunk 0 fine-grained (earliest
            start), chunks 1-3 as one coarse transfer per f-slice."""
            for f in range(NF):
                nc.sync.dma_start(
                    xt_t[:, f, 0, :],
                    xT[f * P:(f + 1) * P, b * S:b * S + CH])
            for f in range(NF):
                nc.sync.dma_start(
                    xt_t[:, f, 1:NJ, :],
                    xT[f * P:(f + 1) * P, b * S + CH:(b + 1) * S])

        def qk_chain(b, j, what, hp):
            """One q/k projection chain (8 MMs) for head pair hp."""
            t0 = j * CH
            base = 0 if what == "q" else HD
            bo_ = 0 if what == "q" else 2
            dst = qp if what == "q" else kp
            ps = mm_ps.tile([P, CH], FP, tag="mm")
            for f in range(NF):
                nc.tensor.matmul(
                    ps[:], w_t[:, f, base + hp * P:base + (hp + 1) * P],
                    xt_t[:, f, j, :],
                    start=(f == 0), stop=(f == NF - 1))
            nc.vector.tensor_scalar_add(
                dst[b][hp][:, t0:t0 + CH], ps[:],
                bias_t[:, bo_ + hp:bo_ + hp + 1])

        def v_pair(b, j, mm0):
            """v projection for two 128-token strips of chunk j."""
            for m in (mm0, mm0 + 1):
                ps = mm_ps.tile([P, CH], FP, tag="mm")
                for f in range(NF):
                    nc.tensor.matmul(
                        ps[:, :HD], xt_t[:, f, j, m * P:(m + 1) * P],
                        w_t[:, f, 2 * HD:3 * HD],
                        start=(f == 0), stop=(f == NF - 1))
                nc.vector.tensor_copy(
                    vtmp[b][:, j * (CH // P) + m, :], ps[:, :HD])
            c0 = j * (CH // P) + mm0
            for h in range(HL):      # scatter these two strips into vo tiles
                nc.sync.dma_start(
                    vo[b][h][:, c0:c0 + 2, :DH],
                    vtmp[b][:, c0:c0 + 2, h * DH:(h + 1) * DH])

        def qkv_chunk(b, j, what):
            """Full chunk projection (both head pairs / all strips)."""
            for i in range(2):
                if what == "v":
                    v_pair(b, j, 2 * i)
                else:
                    qk_chain(b, j, what, i)

        def norm(b, hp, qb, pv):
            """Normalize pv -> ot: rowsums live at partition 64 of each bank.

            Emitted mid-way through the NEXT block so the broadcast matmuls
            never stall the PE queue waiting on the DVE reciprocal."""
            q0 = qb * QB
            nc.vector.tensor_copy(rcpi[DH:DH + 1, :, :], pv[DH:DH + 1, :, :])
            # approx reciprocal mis-executes on single-partition base-64 APs;
            # run it over rows 0..64 (rows 0..63 hold a harmless memset 1.0)
            nc.vector.reciprocal_approx_fast(
                rcps[:, :, :].rearrange("p a b -> p (a b)"),
                rcpi[:, :, :].rearrange("p a b -> p (a b)"))
            nc.vector.tensor_copy(rcp_t[DH:DH + 1, :, :],
                                  rcps[DH:DH + 1, :, :])
            for u in range(2):
                bc = mm_ps.tile([P, QB], FP, tag="mm")
                nc.tensor.matmul(bc[:DH, :], e65[:], rcp_t[:, u, :],
                                 start=True, stop=True)
                rb = stage.tile([DH, QB], FP, tag="rb")
                nc.vector.tensor_copy(rb[:], bc[:DH, :])
                if u == 0:
                    nc.vector.tensor_mul(
                        ot[b][hp][0:DH, q0:q0 + QB], pv[0:DH, u, :], rb[:])
                else:
                    on = stage.tile([DH, QB], FR, tag="on")
                    nc.vector.tensor_mul(on[:], pv[0:DH, u, :], rb[:])
                    nc.sync.dma_start(
                        ot[b][hp][DH:2 * DH, q0:q0 + QB], on[:])

        def attn_block(b, hp, qb, pending, hooks=None):
            """S/exp/PV for one (batch, head pair, 512-query block).

            `pending` is the previous block's deferred norm closure (emitted
            after round 2); returns this block's norm closure."""
            q0 = qb * QB
            pv = pv_ps.tile([DH + 1, 2, QB], FP, tag="pv", name="pv")
            for r in range(NKS):
                if hooks and r in hooks:
                    hooks[r]()
                sp = s_ps.tile([P, 2, QB], FP, tag="s")
                for u in range(2):   # u=0: even head (T0), u=1: odd (T8)
                    lo = u * DH
                    nc.tensor.matmul(
                        sp[:, u, :],
                        kp[b][hp][lo:lo + DH, r * P:(r + 1) * P],
                        qp[b][hp][lo:lo + DH, q0:q0 + QB],
                        start=True, stop=True)
                pt = pt_p.tile([P, 2, QB], FR, tag="pt")
                nc.scalar.activation(pt[:], sp[:], AF.Exp, scale=0.125)
                for u in range(2):
                    nc.tensor.matmul(
                        pv[:, u, :], vo[b][2 * hp + u][:, r, :], pt[:, u, :],
                        start=(r == 0), stop=(r == NKS - 1))
                if r == 2 and pending is not None:
                    pending()
            return lambda: norm(b, hp, qb, pv)

        def proj(b, tq, fos=range(NF)):
            """y^T partial for batch b, 512-token block tq."""
            for fo in fos:
                yp = mm_ps.tile([P, CH], FP, tag="mm")
                for kc in range(2):
                    nc.tensor.matmul(
                        yp[:], wo_t[:, kc, fo * P:(fo + 1) * P],
                        ot[b][kc][:, tq * CH:(tq + 1) * CH],
                        start=(kc == 0), stop=(kc == 1))
                y_sb = stage.tile([P, CH], FP, tag="ysb")
                nc.vector.tensor_copy(y_sb[:], yp[:])
                nc.sync.dma_start(
                    yT[fo * P:(fo + 1) * P,
                       b * S + tq * CH:b * S + (tq + 1) * CH], y_sb[:])

        # ---- emission order ----
        # Prefetch all of batch 0's x, then the minimal lead-in (k/q/v of
        # chunk 0); later chunks feed into block (0,0,0) just in time.
        ldx(0)
        qkv_chunk(0, 0, "k")
        qkv_chunk(0, 0, "q")
        qkv_chunk(0, 0, "v")

        pending = None
        hooks0 = {}
        for c in range(1, NJ):       # k chunk c before round 4c; v likewise
            hooks0[4 * c - 3] = (lambda c=c: qkv_chunk(0, c, "k"))
            hooks0[4 * c - 1] = (lambda c=c: qkv_chunk(0, c, "v"))
        hooks0[13] = lambda: qk_chain(0, 1, "q", 0)
        hooks0[15] = lambda: qk_chain(0, 1, "q", 1)
        pending = attn_block(0, 0, 0, pending, hooks0)

        # attn(b0): remaining b0 q chains + batch-1 chunks 0-1 drip-fed.
        # ldx(1, j) must be emitted after the last b0 reader of xt[j].
        work = [lambda: qk_chain(0, 2, "q", 0),
                lambda: qk_chain(0, 2, "q", 1),
                lambda: qk_chain(0, 3, "q", 0),
                lambda: (qk_chain(0, 3, "q", 1), ldx(1))]
        for c in range(2):
            work += [lambda c=c: qk_chain(1, c, "k", 0),
                     lambda c=c: qk_chain(1, c, "k", 1),
                     lambda c=c: v_pair(1, c, 0),
                     lambda c=c: v_pair(1, c, 2)]
        work[6:6] = [lambda: qk_chain(1, 0, "q", 0),
                     lambda: qk_chain(1, 0, "q", 1)]
        wi = 0
        for blk, (hp, qb) in enumerate(
                [(hp, qb) for hp in range(2) for qb in range(NQB)]):
            if blk == 0:
                continue
            hooks = {}
            for r in (2, 7, 12):
                if wi < len(work):
                    hooks[r] = work[wi]
                    wi += 1
            pending = attn_block(0, hp, qb, pending, hooks)
        while wi < len(work):
            work[wi]()
            wi += 1

        # attn(b1) qb-outer; b1 k/v chunks 2-3 feed progressively into the
        # first two blocks; q chunks and both proj batches via hooks
        for qb in range(NQB):
            if qb == 0:
                hooks = {1: lambda: qk_chain(1, 2, "k", 0),
                         3: lambda: qk_chain(1, 2, "k", 1),
                         5: lambda: v_pair(1, 2, 0),
                         7: lambda: v_pair(1, 2, 2),
                         9: lambda: qk_chain(1, 3, "k", 0),
                         10: lambda: qk_chain(1, 3, "k", 1),
                         11: lambda: v_pair(1, 3, 0),
                         13: lambda: v_pair(1, 3, 2)}
            else:
                hooks = {5: (lambda qb=qb: proj(1, qb - 1, range(0, 4))),
                         10: (lambda qb=qb: proj(1, qb - 1, range(4, NF)))}
            pending = attn_block(1, 0, qb, pending, hooks)
            hooks = {5: (lambda qb=qb: proj(0, qb, range(0, 4))),
                     10: (lambda qb=qb: proj(0, qb, range(4, NF)))}
            if qb < NQB - 1:
                hooks[1] = (lambda qb=qb: qk_chain(1, qb + 1, "q", 0))
                hooks[3] = (lambda qb=qb: qk_chain(1, qb + 1, "q", 1))
            pending = attn_block(1, 1, qb, pending, hooks)
        pending()
        proj(1, NQB - 1)

    nc.compile()
    return nc


def build():
    if "nc" not in _cache:
        _cache["nc"] = _build()
    return _cache["nc"]


def make_in_maps(x, Wqkv, bqkv, Wo):
    import ml_dtypes
    mmdt = ml_dtypes.bfloat16
    x = np.ascontiguousarray(np.asarray(x, np.float32))
    Wqkv = np.asarray(Wqkv, np.float32)
    bqkv = np.asarray(bqkv, np.float32)
    Wo = np.asarray(Wo, np.float32)
    in_maps = []
    for c in range(NCORES):
        g, t = divmod(c, TP)
        xTc = np.ascontiguousarray(
            x[g * BL:(g + 1) * BL].reshape(TOK, D).T.astype(mmdt))
        wc = np.ascontiguousarray(np.concatenate(
            [Wqkv[:, i * D + t * HD:i * D + (t + 1) * HD] for i in range(3)],
            axis=1).astype(mmdt))
        bqkc = np.ascontiguousarray(np.concatenate(
            [bqkv[t * HD:(t + 1) * HD],
             bqkv[D + t * HD:D + (t + 1) * HD]]).reshape(2 * HD, 1))
        woc = np.ascontiguousarray(Wo[t * HD:(t + 1) * HD, :].astype(mmdt))
        in_maps.append({"xT": xTc, "w": wc, "bqk": bqkc, "wo": woc})
    return in_maps


LAST_EXEC_NS = None


def kernel(x, Wqkv, bqkv, Wo, bo):
    global LAST_EXEC_NS
    from concourse import bass_utils

    nc = build()
    in_maps = make_in_maps(x, Wqkv, bqkv, Wo)
    res = bass_utils.run_bass_kernel_spmd(
        nc, in_maps, core_ids=list(range(NCORES)))
    LAST_EXEC_NS = res.exec_time_ns
    outs = [r["yT"] for r in res.results]

    Wo = np.asarray(Wo, np.float32)
    bo = np.asarray(bo, np.float32)
    bqkv = np.asarray(bqkv, np.float32)
    hb = bo + np.asarray(bqkv[2 * D:3 * D], np.float32) @ Wo

    halves = []
    for g in range(DP):
        acc = outs[g * TP].astype(np.float32)
        for t in range(1, TP):
            acc = acc + outs[g * TP + t]
        halves.append(acc.T)            # [TOK, D]
    y = np.concatenate(halves, axis=0) + hb[None, :]
    return np.ascontiguousarray(y.reshape(B, S, D).astype(np.float32))


# revision 29
# speedup vs baseline: 1.0052x; 1.0052x over previous
"""Multi-head attention forward, tensor-parallel over 8 TRN2 NeuronCores.

Problem: x[4,2048,1024], Wqkv[1024,3072], bqkv[3072], Wo[1024,1024], bo[1024]
  qkv = x @ Wqkv + bqkv ; 16 heads, d_head 64 ; softmax(QK^T/8) V ; out proj.

Sharding: DP=2 over batch (2 batches/core) x TP=4 over heads (4 heads/core).
Each core computes a partial y^T (its heads' contribution, transposed); the
host sums partials within each batch group, adds biases, and transposes.

Device dataflow (v2 — fully SBUF-resident, head-pair row tiling):
  qT,kT = (W_{q,k}^T x^T + b)   [128, 2048] per (batch, head-pair):
                                partitions 0-63 = even head, 64-127 = odd head
  v     = x W_v                 [128 tok, 256] strips -> vtmp -> vo per head
  S^T   = K Q^T per (b,hp,qb,strip): TWO heads concurrently as 64-row PE
          tiles T0/T8 (K=64, operands live on partition halves) -> one
          2-bank PSUM tile [128, 2, 512]
  P^T   = exp(S^T/8)            one ACT call, N=1024, bf16 out
  O^T|s = [V|1]^T P^T           per head, [65, 512] PSUM accumulated over 16
                                strips; row 64 = rowsum
  norm  : reciprocal_approx_fast on rowsums, f32r broadcast matmul (e65),
          DVE mul -> ot tiles [128, 2048] (odd head shifted via SBUF DMA)
  y^T  += Wo_part^T O_n^T       [1024, 2048] partial per batch, summed on host
"""

import sys

if "/opt/trn_rl_repo" not in sys.path:
    sys.path.insert(0, "/opt/trn_rl_repo")

import numpy as np

B, S, D = 4, 2048, 1024
H, DH = 16, 64
NCORES = 8
DP, TP = 2, 4
BL = B // DP            # 2 local batches
TOK = BL * S            # 4096 local tokens
HL = H // TP            # 4 local heads
HD = HL * DH            # 256 local head dims
P = 128
NF = D // P             # 8 contraction chunks
CH = 512                # token chunk for projections
NJ = S // CH            # 4 chunks per batch
NKS = S // P            # 16 k-strips per (batch, head)
QB = 512                # q block (one PSUM bank)
NQB = S // QB           # 4 q blocks per head

_cache = {}


def _build():
    import concourse.bass as bass
    import concourse.tile as tile
    from concourse import bacc, mybir
    from contextlib import ExitStack

    FP = mybir.dt.float32
    FR = mybir.dt.bfloat16
    F32R = mybir.dt.float32r
    AF = mybir.ActivationFunctionType

    nc = bacc.Bacc("TRN2", target_bir_lowering=False, debug=False,
                   num_devices=NCORES)

    xT = nc.dram_tensor("xT", [D, TOK], FR, kind="ExternalInput").ap()
    w = nc.dram_tensor("w", [D, 3 * HD], FR, kind="ExternalInput").ap()
    bqk = nc.dram_tensor("bqk", [2 * HD, 1], FP, kind="ExternalInput").ap()
    wo = nc.dram_tensor("wo", [HD, D], FR, kind="ExternalInput").ap()
    yT = nc.dram_tensor("yT", [D, TOK], FP, kind="ExternalOutput").ap()

    with tile.TileContext(nc) as tc, ExitStack() as ctx:
        konst = ctx.enter_context(tc.tile_pool(name="konst", bufs=1))
        xt_p = ctx.enter_context(tc.tile_pool(name="xt", bufs=1))
        qp_p = ctx.enter_context(tc.tile_pool(name="qp", bufs=1))
        kp_p = ctx.enter_context(tc.tile_pool(name="kp", bufs=1))
        vt_p = ctx.enter_context(tc.tile_pool(name="vt", bufs=1))
        vo_p = ctx.enter_context(tc.tile_pool(name="vo", bufs=1))
        ot_p = ctx.enter_context(tc.tile_pool(name="ot", bufs=1))
        pt_p = ctx.enter_context(tc.tile_pool(name="pt", bufs=12))
        stage = ctx.enter_context(tc.tile_pool(name="stage", bufs=4))
        s_ps = ctx.enter_context(
            tc.tile_pool(name="sps", bufs=2, space="PSUM"))
        pv_ps = ctx.enter_context(
            tc.tile_pool(name="pvps", bufs=1, space="PSUM"))
        mm_ps = ctx.enter_context(
            tc.tile_pool(name="mmps", bufs=2, space="PSUM"))

        # ---- constants resident in SBUF ----
        w_t = konst.tile([P, NF, 3 * HD], FR, tag="w")
        for f in range(NF):
            nc.sync.dma_start(w_t[:, f, :], w[f * P:(f + 1) * P, :])
        wo_t = konst.tile([P, 2, D], FR, tag="wo")
        for kc in range(2):
            nc.sync.dma_start(wo_t[:, kc, :], wo[kc * P:(kc + 1) * P, :])
        bias_t = konst.tile([P, 4], FP, tag="bias")
        for o in range(4):
            nc.sync.dma_start(bias_t[:, o:o + 1], bqk[o * P:(o + 1) * P, :])
        # e65: selects row 64 (the rowsum) in the broadcast matmul (f32r for
        # full PE rate; memset through an fp32 bitcast view)
        e65 = konst.tile([DH + 1, DH], FR, tag="e65")
        nc.gpsimd.memset(e65[:], 0.0)
        nc.gpsimd.memset(e65[DH:DH + 1, :], 1.0)
        # reciprocal staging: row 64 written per norm event; rows 0..63 are
        # constant (multiplied by e65's zeros) but must stay finite
        rcp_t = konst.tile([DH + 1, 2, QB], FR, tag="rcp")
        nc.gpsimd.memset(rcp_t[:], 1.0)
        rcps = konst.tile([DH + 1, 2, QB], FP, tag="rcps")
        rcpi = konst.tile([DH + 1, 2, QB], FP, tag="rcpi")
        nc.gpsimd.memset(rcpi[:], 1.0)
        # fp32 ones row used to fill the vo ones column
        ones16 = konst.tile([P, NKS], FP, tag="ones16")
        nc.gpsimd.memset(ones16[:], 1.0)
        # ACT exp table warm-up (first Exp pays ~2.7us table DMA)
        warm = konst.tile([1, 4], FR, tag="warm")
        nc.scalar.activation(warm[:], bias_t[0:1, :], AF.Exp, scale=0.125)

        # ---- persistent SBUF tensors ----
        # x for one batch (reused batch 1 over batch 0 via WAR deps)
        xt_t = xt_p.tile([P, NF, NJ, CH], FR, tag="xt", name="xt_t")

        # q/k: [128, 2048] per (batch, head-pair); partitions 0-63 even head
        qp = [[qp_p.tile([P, S], FR, tag=f"qp{b}_{hp}", name=f"qp{b}_{hp}")
               for hp in range(2)] for b in range(BL)]
        kp = [[kp_p.tile([P, S], FR, tag=f"kp{b}_{hp}", name=f"kp{b}_{hp}")
               for hp in range(2)] for b in range(BL)]
        # v staging [128 tok, strip, 256 feat] and per-head [V|1] tiles
        vtmp = [vt_p.tile([P, NKS, HD], FR, tag=f"vt{b}", name=f"vt{b}")
                for b in range(BL)]
        vo = [[vo_p.tile([P, NKS, DH + 1], FR, tag=f"vo{b}_{h}",
                         name=f"vo{b}_{h}") for h in range(HL)]
              for b in range(BL)]
        for b in range(BL):
            for h in range(HL):
                nc.vector.tensor_copy(vo[b][h][:, :, DH], ones16[:])
        # normalized O^T, 2 heads stacked per tile
        ot = [[ot_p.tile([P, S], FR, tag=f"ot{b}_{hp}", name=f"ot{b}_{hp}")
               for hp in range(2)] for b in range(BL)]

        def ldx(b, j=None):
            """Queue x DMAs for batch b: chunk 0 fine-grained (earliest
            start), chunks 1-3 as one coarse transfer per f-slice."""
            for f in range(NF):
                nc.sync.dma_start(
                    xt_t[:, f, 0, :],
                    xT[f * P:(f + 1) * P, b * S:b * S + CH])
            for f in range(NF):
                nc.sync.dma_start(
                    xt_t[:, f, 1:NJ, :],
                    xT[f * P:(f + 1) * P, b * S + CH:(b + 1) * S])

        def qk_chain(b, j, what, hp):
            """One q/k projection chain (8 MMs) for head pair hp."""
            t0 = j * CH
            base = 0 if what == "q" else HD
            bo_ = 0 if what == "q" else 2
            dst = qp if what == "q" else kp
            ps = mm_ps.tile([P, CH], FP, tag="mm")
            for f in range(NF):
                nc.tensor.matmul(
                    ps[:], w_t[:, f, base + hp * P:base + (hp + 1) * P],
                    xt_t[:, f, j, :],
                    start=(f == 0), stop=(f == NF - 1))
            nc.vector.tensor_scalar_add(
                dst[b][hp][:, t0:t0 + CH], ps[:],
                bias_t[:, bo_ + hp:bo_ + hp + 1])

        def v_pair(b, j, mm0):
            """v projection for two 128-token strips of chunk j."""
            for m in (mm0, mm0 + 1):
                ps = mm_ps.tile([P, CH], FP, tag="mm")
                for f in range(NF):
                    nc.tensor.matmul(
                        ps[:, :HD], xt_t[:, f, j, m * P:(m + 1) * P],
                        w_t[:, f, 2 * HD:3 * HD],
                        start=(f == 0), stop=(f == NF - 1))
                nc.vector.tensor_copy(
                    vtmp[b][:, j * (CH // P) + m, :], ps[:, :HD])
            c0 = j * (CH // P) + mm0
            for h in range(HL):      # scatter these two strips into vo tiles
                nc.sync.dma_start(
                    vo[b][h][:, c0:c0 + 2, :DH],
                    vtmp[b][:, c0:c0 + 2, h * DH:(h + 1) * DH])

        def qkv_chunk(b, j, what):
            """Full chunk projection (both head pairs / all strips)."""
            for i in range(2):
                if what == "v":
                    v_pair(b, j, 2 * i)
                else:
                    qk_chain(b, j, what, i)

        def norm(b, hp, qb, pv):
            """Normalize pv -> ot: rowsums live at partition 64 of each bank.

            Emitted mid-way through the NEXT block so the broadcast matmuls
            never stall the PE queue waiting on the DVE reciprocal."""
            q0 = qb * QB
            nc.vector.tensor_copy(rcpi[DH:DH + 1, :, :], pv[DH:DH + 1, :, :])
            # approx reciprocal mis-executes on single-partition base-64 APs;
            # run it over rows 0..64 (rows 0..63 hold a harmless memset 1.0)
            nc.vector.reciprocal_approx_fast(
                rcps[:, :, :].rearrange("p a b -> p (a b)"),
                rcpi[:, :, :].rearrange("p a b -> p (a b)"))
            nc.vector.tensor_copy(rcp_t[DH:DH + 1, :, :],
                                  rcps[DH:DH + 1, :, :])
            for u in range(2):
                bc = mm_ps.tile([P, QB], FP, tag="mm")
                nc.tensor.matmul(bc[:DH, :], e65[:], rcp_t[:, u, :],
                                 start=True, stop=True)
                rb = stage.tile([DH, QB], FP, tag="rb")
                nc.vector.tensor_copy(rb[:], bc[:DH, :])
                if u == 0:
                    nc.vector.tensor_mul(
                        ot[b][hp][0:DH, q0:q0 + QB], pv[0:DH, u, :], rb[:])
                else:
                    on = stage.tile([DH, QB], FR, tag="on")
                    nc.vector.tensor_mul(on[:], pv[0:DH, u, :], rb[:])
                    nc.sync.dma_start(
                        ot[b][hp][DH:2 * DH, q0:q0 + QB], on[:])

        def attn_block(b, hp, qb, pending, hooks=None):
            """S/exp/PV for one (batch, head pair, 512-query block).

            `pending` is the previous block's deferred norm closure (emitted
            after round 2); returns this block's norm closure."""
            q0 = qb * QB
            pv = pv_ps.tile([DH + 1, 2, QB], FP, tag="pv", name="pv")
            for r in range(NKS):
                if hooks and r in hooks:
                    hooks[r]()
                sp = s_ps.tile([P, 2, QB], FP, tag="s")
                for u in range(2):   # u=0: even head (T0), u=1: odd (T8)
                    lo = u * DH
                    nc.tensor.matmul(
                        sp[:, u, :],
                        kp[b][hp][lo:lo + DH, r * P:(r + 1) * P],
                        qp[b][hp][lo:lo + DH, q0:q0 + QB],
                        start=True, stop=True)
                pt = pt_p.tile([P, 2, QB], FR, tag="pt")
                nc.scalar.activation(pt[:], sp[:], AF.Exp, scale=0.125)
                for u in range(2):
                    nc.tensor.matmul(
                        pv[:, u, :], vo[b][2 * hp + u][:, r, :], pt[:, u, :],
                        start=(r == 0), stop=(r == NKS - 1))
                if r == 2 and pending is not None:
                    pending()
            return lambda: norm(b, hp, qb, pv)

        def proj(b, tq, fos=range(NF)):
            """y^T partial for batch b, 512-token block tq."""
            for fo in fos:
                yp = mm_ps.tile([P, CH], FP, tag="mm")
                for kc in range(2):
                    nc.tensor.matmul(
                        yp[:], wo_t[:, kc, fo * P:(fo + 1) * P],
                        ot[b][kc][:, tq * CH:(tq + 1) * CH],
                        start=(kc == 0), stop=(kc == 1))
                y_sb = stage.tile([P, CH], FP, tag="ysb")
                nc.vector.tensor_copy(y_sb[:], yp[:])
                nc.sync.dma_start(
                    yT[fo * P:(fo + 1) * P,
                       b * S + tq * CH:b * S + (tq + 1) * CH], y_sb[:])

        # ---- emission order ----
        # Prefetch all of batch 0's x, then the minimal lead-in (k/q/v of
        # chunk 0); later chunks feed into block (0,0,0) just in time.
        ldx(0)
        qkv_chunk(0, 0, "k")
        qkv_chunk(0, 0, "q")
        qkv_chunk(0, 0, "v")

        pending = None
        hooks0 = {}
        for c in range(1, NJ):       # k chunk c before round 4c; v likewise
            hooks0[4 * c - 3] = (lambda c=c: qkv_chunk(0, c, "k"))
            hooks0[4 * c - 1] = (lambda c=c: qkv_chunk(0, c, "v"))
        hooks0[13] = lambda: qk_chain(0, 1, "q", 0)
        hooks0[15] = lambda: qk_chain(0, 1, "q", 1)
        pending = attn_block(0, 0, 0, pending, hooks0)

        # attn(b0): remaining b0 q chains + batch-1 chunks 0-1 drip-fed.
        # ldx(1, j) must be emitted after the last b0 reader of xt[j].
        work = [lambda: qk_chain(0, 2, "q", 0),
                lambda: qk_chain(0, 2, "q", 1),
                lambda: qk_chain(0, 3, "q", 0),
                lambda: (qk_chain(0, 3, "q", 1), ldx(1))]
        for c in range(2):
            work += [lambda c=c: qk_chain(1, c, "k", 0),
                     lambda c=c: qk_chain(1, c, "k", 1),
                     lambda c=c: v_pair(1, c, 0),
                     lambda c=c: v_pair(1, c, 2)]
        work[6:6] = [lambda: qk_chain(1, 0, "q", 0),
                     lambda: qk_chain(1, 0, "q", 1)]
        wi = 0
        for blk, (hp, qb) in enumerate(
                [(hp, qb) for hp in range(2) for qb in range(NQB)]):
            if blk == 0:
                continue
            hooks = {}
            for r in (2, 7, 12):
                if wi < len(work):
                    hooks[r] = work[wi]
                    wi += 1
            pending = attn_block(0, hp, qb, pending, hooks)
        while wi < len(work):
            work[wi]()
            wi += 1

        # attn(b1) qb-outer; b1 k/v chunks 2-3 feed progressively into the
        # first two blocks; q chunks and both proj batches via hooks
        for qb in range(NQB):
            if qb == 0:
                hooks = {1: lambda: qk_chain(1, 2, "k", 0),
                         3: lambda: qk_chain(1, 2, "k", 1),
                         5: lambda: v_pair(1, 2, 0),
                         7: lambda: v_pair(1, 2, 2),
                         9: lambda: qk_chain(1, 3, "k", 0),
                         10: lambda: qk_chain(1, 3, "k", 1),
                         11: lambda: v_pair(1, 3, 0),
                         13: lambda: v_pair(1, 3, 2)}
            else:
                hooks = {5: (lambda qb=qb: proj(1, qb - 1, range(0, 4))),
                         10: (lambda qb=qb: proj(1, qb - 1, range(4, NF)))}
            pending = attn_block(1, 0, qb, pending, hooks)
            hooks = {5: (lambda qb=qb: proj(0, qb, range(0, 4))),
                     10: (lambda qb=qb: proj(0, qb, range(4, NF)))}
            if qb < NQB - 1:
                hooks[1] = (lambda qb=qb: qk_chain(1, qb + 1, "q", 0))
                hooks[3] = (lambda qb=qb: qk_chain(1, qb + 1, "q", 1))
            pending = attn_block(1, 1, qb, pending, hooks)
        pending()
        proj(1, NQB - 1)

    nc.compile()
    return nc


def build():
    if "nc" not in _cache:
        _cache["nc"] = _build()
    return _cache["nc"]


def make_in_maps(x, Wqkv, bqkv, Wo):
    import ml_dtypes
    mmdt = ml_dtypes.bfloat16
    x = np.ascontiguousarray(np.asarray(x, np.float32))
    Wqkv = np.asarray(Wqkv, np.float32)
    bqkv = np.asarray(bqkv, np.float32)
    Wo = np.asarray(Wo, np.float32)
    in_maps = []
    for c in range(NCORES):
        g, t = divmod(c, TP)
        xTc = np.ascontiguousarray(
            x[g * BL:(g + 1) * BL].reshape(TOK, D).T.astype(mmdt))
        wc = np.ascontiguousarray(np.concatenate(
            [Wqkv[:, i * D + t * HD:i * D + (t + 1) * HD] for i in range(3)],
            axis=1).astype(mmdt))
        bqkc = np.ascontiguousarray(np.concatenate(
            [bqkv[t * HD:(t + 1) * HD],
             bqkv[D + t * HD:D + (t + 1) * HD]]).reshape(2 * HD, 1))
        woc = np.ascontiguousarray(Wo[t * HD:(t + 1) * HD, :].astype(mmdt))
        in_maps.append({"xT": xTc, "w": wc, "bqk": bqkc, "wo": woc})
    return in_maps


LAST_EXEC_NS = None


def kernel(x, Wqkv, bqkv, Wo, bo):
    global LAST_EXEC_NS
    from concourse import bass_utils

    nc = build()
    in_maps = make_in_maps(x, Wqkv, bqkv, Wo)
    res = bass_utils.run_bass_kernel_spmd(
        nc, in_maps, core_ids=list(range(NCORES)))
    LAST_EXEC_NS = res.exec_time_ns
    outs = [r["yT"] for r in res.results]

    Wo = np.asarray(Wo, np.float32)
    bo = np.asarray(bo, np.float32)
    bqkv = np.asarray(bqkv, np.float32)
    hb = bo + np.asarray(bqkv[2 * D:3 * D], np.float32) @ Wo

    halves = []
    for g in range(DP):
        acc = outs[g * TP].astype(np.float32)
        for t in range(1, TP):
            acc = acc + outs[g * TP + t]
        halves.append(acc.T)            # [TOK, D]
    y = np.concatenate(halves, axis=0) + hb[None, :]
    return np.ascontiguousarray(y.reshape(B, S, D).astype(np.float32))


# revision 33
# speedup vs baseline: 1.0057x; 1.0006x over previous
"""Multi-head attention forward, tensor-parallel over 8 TRN2 NeuronCores.

Problem: x[4,2048,1024], Wqkv[1024,3072], bqkv[3072], Wo[1024,1024], bo[1024]
  qkv = x @ Wqkv + bqkv ; 16 heads, d_head 64 ; softmax(QK^T/8) V ; out proj.

Sharding: DP=2 over batch (2 batches/core) x TP=4 over heads (4 heads/core).
Each core computes a partial y^T (its heads' contribution, transposed); the
host sums partials within each batch group, adds biases, and transposes.

Device dataflow (fully SBUF-resident, head-pair row tiling):
  qT,kT = (W_{q,k}^T x^T + b)   [128, 2048] per (batch, head-pair):
                                partitions 0-63 = even head, 64-127 = odd head
  v     = x W_v                 [128 tok, 256] strips -> vtmp -> vo per head
  S^T   = K Q^T per (b,hp,qb,strip): TWO heads concurrently as 64-row PE
          tiles T0/T8 (K=64, operands live on partition halves) -> one
          2-bank PSUM tile [128, 2, 512]
  P^T   = exp(S^T/8)            one ACT call, N=1024, bf16 out
  O^T|s = [V|1]^T P^T           per head, [65, 512] PSUM accumulated over 16
                                strips; row 64 = rowsum
  norm  : reciprocal_approx_fast on rowsums (bug workarounds: 2D base-0 AP
          only), bf16 e65 broadcast matmul, DVE mul -> ot tiles [128, 2048]
          (odd head shifted via SBUF->SBUF DMA)
  y^T  += Wo_part^T O_n^T       [1024, 2048] partial per batch, summed on host

Scheduling: each block's normalization is software-pipelined into round 2 of
the NEXT block (never stalls the PE queue on the DVE reciprocal); projection
chains for the other batch and the output projection are drip-fed into
attention blocks through per-round hooks so the PE never idles long enough
to re-throttle (HAM) and the scalar engine stays saturated.
"""

import sys

if "/opt/trn_rl_repo" not in sys.path:
    sys.path.insert(0, "/opt/trn_rl_repo")

import numpy as np

B, S, D = 4, 2048, 1024
H, DH = 16, 64
NCORES = 8
DP, TP = 2, 4
BL = B // DP            # 2 local batches
TOK = BL * S            # 4096 local tokens
HL = H // TP            # 4 local heads
HD = HL * DH            # 256 local head dims
P = 128
NF = D // P             # 8 contraction chunks
CH = 512                # token chunk for projections
NJ = S // CH            # 4 chunks per batch
NKS = S // P            # 16 k-strips per (batch, head)
QB = 512                # q block (one PSUM bank)
NQB = S // QB           # 4 q blocks per head

_cache = {}


def _build():
    import concourse.bass as bass
    import concourse.tile as tile
    from concourse import bacc, mybir
    from contextlib import ExitStack

    FP = mybir.dt.float32
    FR = mybir.dt.bfloat16
    F32R = mybir.dt.float32r
    AF = mybir.ActivationFunctionType

    nc = bacc.Bacc("TRN2", target_bir_lowering=False, debug=False,
                   num_devices=NCORES)

    xT = nc.dram_tensor("xT", [D, TOK], FR, kind="ExternalInput").ap()
    w = nc.dram_tensor("w", [D, 3 * HD], FR, kind="ExternalInput").ap()
    bqk = nc.dram_tensor("bqk", [2 * HD, 1], FP, kind="ExternalInput").ap()
    wo = nc.dram_tensor("wo", [HD, D], FR, kind="ExternalInput").ap()
    yT = nc.dram_tensor("yT", [D, TOK], FP, kind="ExternalOutput").ap()

    with tile.TileContext(nc) as tc, ExitStack() as ctx:
        konst = ctx.enter_context(tc.tile_pool(name="konst", bufs=1))
        xt_p = ctx.enter_context(tc.tile_pool(name="xt", bufs=1))
        qp_p = ctx.enter_context(tc.tile_pool(name="qp", bufs=1))
        kp_p = ctx.enter_context(tc.tile_pool(name="kp", bufs=1))
        vt_p = ctx.enter_context(tc.tile_pool(name="vt", bufs=1))
        vo_p = ctx.enter_context(tc.tile_pool(name="vo", bufs=1))
        ot_p = ctx.enter_context(tc.tile_pool(name="ot", bufs=1))
        pt_p = ctx.enter_context(tc.tile_pool(name="pt", bufs=12))
        stage = ctx.enter_context(tc.tile_pool(name="stage", bufs=4))
        s_ps = ctx.enter_context(
            tc.tile_pool(name="sps", bufs=2, space="PSUM"))
        pv_ps = ctx.enter_context(
            tc.tile_pool(name="pvps", bufs=1, space="PSUM"))
        mm_ps = ctx.enter_context(
            tc.tile_pool(name="mmps", bufs=2, space="PSUM"))

        # ---- constants resident in SBUF ----
        # (w DMAs are interleaved with the first x-chunk loads below so the
        # first k-projection chain can start as early as possible)
        w_t = konst.tile([P, NF, 3 * HD], FR, tag="w")
        wo_t = konst.tile([P, 2, D], FR, tag="wo")
        bias_t = konst.tile([P, 4], FP, tag="bias")
        for o in range(4):
            nc.sync.dma_start(bias_t[:, o:o + 1], bqk[o * P:(o + 1) * P, :])
        # e65: selects row 64 (the rowsum) in the broadcast matmul (f32r for
        # full PE rate; memset through an fp32 bitcast view)
        e65 = konst.tile([DH + 1, DH], FR, tag="e65")
        nc.gpsimd.memset(e65[:], 0.0)
        nc.gpsimd.memset(e65[DH:DH + 1, :], 1.0)
        # reciprocal staging: row 64 written per norm event; rows 0..63 are
        # constant (multiplied by e65's zeros) but must stay finite
        rcp_t = konst.tile([DH + 1, 2, QB], FR, tag="rcp")
        nc.gpsimd.memset(rcp_t[:], 1.0)
        rcps = konst.tile([DH + 1, 2, QB], FP, tag="rcps")
        rcpi = konst.tile([DH + 1, 2, QB], FP, tag="rcpi")
        nc.gpsimd.memset(rcpi[:], 1.0)
        # fp32 ones row used to fill the vo ones column
        ones16 = konst.tile([P, NKS], FP, tag="ones16")
        nc.gpsimd.memset(ones16[:], 1.0)
        # ACT exp table warm-up (first Exp pays ~2.7us table DMA)
        warm = konst.tile([1, 4], FR, tag="warm")
        nc.scalar.activation(warm[:], bias_t[0:1, :], AF.Exp, scale=0.125)

        # ---- persistent SBUF tensors ----
        # x for one batch (reused batch 1 over batch 0 via WAR deps)
        xt_t = xt_p.tile([P, NF, NJ, CH], FR, tag="xt", name="xt_t")

        # q/k: [128, 2048] per (batch, head-pair); partitions 0-63 even head
        qp = [[qp_p.tile([P, S], FR, tag=f"qp{b}_{hp}", name=f"qp{b}_{hp}")
               for hp in range(2)] for b in range(BL)]
        kp = [[kp_p.tile([P, S], FR, tag=f"kp{b}_{hp}", name=f"kp{b}_{hp}")
               for hp in range(2)] for b in range(BL)]
        # v staging [128 tok, strip, 256 feat] and per-head [V|1] tiles
        vtmp = [vt_p.tile([P, NKS, HD], FR, tag=f"vt{b}", name=f"vt{b}")
                for b in range(BL)]
        vo = [[vo_p.tile([P, NKS, DH + 1], FR, tag=f"vo{b}_{h}",
                         name=f"vo{b}_{h}") for h in range(HL)]
              for b in range(BL)]
        for b in range(BL):
            for h in range(HL):
                nc.vector.tensor_copy(vo[b][h][:, :, DH], ones16[:])
        # normalized O^T, 2 heads stacked per tile
        ot = [[ot_p.tile([P, S], FR, tag=f"ot{b}_{hp}", name=f"ot{b}_{hp}")
               for hp in range(2)] for b in range(BL)]

        def ldx(b, preamble=False):
            """Queue x DMAs for batch b: chunk 0 fine-grained (earliest
            start), chunks 1-3 as one coarse transfer per f-slice. In the
            preamble, pair each chunk-0 slice with its w slice so the first
            k chain is fed per-f."""
            for f in range(NF):
                nc.sync.dma_start(
                    xt_t[:, f, 0, :],
                    xT[f * P:(f + 1) * P, b * S:b * S + CH])
                if preamble:
                    nc.sync.dma_start(w_t[:, f, :], w[f * P:(f + 1) * P, :])
            for f in range(NF):
                nc.sync.dma_start(
                    xt_t[:, f, 1:NJ, :],
                    xT[f * P:(f + 1) * P, b * S + CH:(b + 1) * S])
            if preamble:
                for kc in range(2):
                    nc.sync.dma_start(wo_t[:, kc, :],
                                      wo[kc * P:(kc + 1) * P, :])

        def qk_chain(b, j, what, hp):
            """One q/k projection chain (8 MMs) for head pair hp."""
            t0 = j * CH
            base = 0 if what == "q" else HD
            bo_ = 0 if what == "q" else 2
            dst = qp if what == "q" else kp
            ps = mm_ps.tile([P, CH], FP, tag="mm")
            for f in range(NF):
                nc.tensor.matmul(
                    ps[:], w_t[:, f, base + hp * P:base + (hp + 1) * P],
                    xt_t[:, f, j, :],
                    start=(f == 0), stop=(f == NF - 1))
            nc.vector.tensor_scalar_add(
                dst[b][hp][:, t0:t0 + CH], ps[:],
                bias_t[:, bo_ + hp:bo_ + hp + 1])

        def v_pair(b, j, mm0):
            """v projection for two 128-token strips of chunk j."""
            for m in (mm0, mm0 + 1):
                ps = mm_ps.tile([P, CH], FP, tag="mm")
                for f in range(NF):
                    nc.tensor.matmul(
                        ps[:, :HD], xt_t[:, f, j, m * P:(m + 1) * P],
                        w_t[:, f, 2 * HD:3 * HD],
                        start=(f == 0), stop=(f == NF - 1))
                nc.vector.tensor_copy(
                    vtmp[b][:, j * (CH // P) + m, :], ps[:, :HD])
            c0 = j * (CH // P) + mm0
            for h in range(HL):      # scatter these two strips into vo tiles
                nc.sync.dma_start(
                    vo[b][h][:, c0:c0 + 2, :DH],
                    vtmp[b][:, c0:c0 + 2, h * DH:(h + 1) * DH])

        def qkv_chunk(b, j, what):
            """Full chunk projection (both head pairs / all strips)."""
            for i in range(2):
                if what == "v":
                    v_pair(b, j, 2 * i)
                else:
                    qk_chain(b, j, what, i)

        def norm(b, hp, qb, pv):
            """Normalize pv -> ot: rowsums live at partition 64 of each bank.

            Emitted mid-way through the NEXT block so the broadcast matmuls
            never stall the PE queue waiting on the DVE reciprocal."""
            q0 = qb * QB
            nc.vector.tensor_copy(rcpi[DH:DH + 1, :, :], pv[DH:DH + 1, :, :])
            # approx reciprocal mis-executes on single-partition base-64 APs;
            # run it over rows 0..64 (rows 0..63 hold a harmless memset 1.0)
            nc.vector.reciprocal_approx_fast(
                rcps[:, :, :].rearrange("p a b -> p (a b)"),
                rcpi[:, :, :].rearrange("p a b -> p (a b)"))
            nc.vector.tensor_copy(rcp_t[DH:DH + 1, :, :],
                                  rcps[DH:DH + 1, :, :])
            for u in range(2):
                bc = mm_ps.tile([P, QB], FP, tag="mm")
                nc.tensor.matmul(bc[:DH, :], e65[:], rcp_t[:, u, :],
                                 start=True, stop=True)
                rb = stage.tile([DH, QB], FP, tag="rb")
                nc.vector.tensor_copy(rb[:], bc[:DH, :])
                if u == 0:
                    nc.vector.tensor_mul(
                        ot[b][hp][0:DH, q0:q0 + QB], pv[0:DH, u, :], rb[:])
                else:
                    on = stage.tile([DH, QB], FR, tag="on")
                    nc.vector.tensor_mul(on[:], pv[0:DH, u, :], rb[:])
                    nc.sync.dma_start(
                        ot[b][hp][DH:2 * DH, q0:q0 + QB], on[:])

        def attn_block(b, hp, qb, pending, hooks=None):
            """S/exp/PV for one (batch, head pair, 512-query block).

            `pending` is the previous block's deferred norm closure (emitted
            after round 2); returns this block's norm closure."""
            q0 = qb * QB
            pv = pv_ps.tile([DH + 1, 2, QB], FP, tag="pv", name="pv")
            for r in range(NKS):
                if hooks and r in hooks:
                    hooks[r]()
                sp = s_ps.tile([P, 2, QB], FP, tag="s")
                for u in range(2):   # u=0: even head (T0), u=1: odd (T8)
                    lo = u * DH
                    nc.tensor.matmul(
                        sp[:, u, :],
                        kp[b][hp][lo:lo + DH, r * P:(r + 1) * P],
                        qp[b][hp][lo:lo + DH, q0:q0 + QB],
                        start=True, stop=True)
                pt = pt_p.tile([P, 2, QB], FR, tag="pt")
                nc.scalar.activation(pt[:], sp[:], AF.Exp, scale=0.125)
                for u in range(2):
                    nc.tensor.matmul(
                        pv[:, u, :], vo[b][2 * hp + u][:, r, :], pt[:, u, :],
                        start=(r == 0), stop=(r == NKS - 1))
                if r == 2 and pending is not None:
                    pending()
            return lambda: norm(b, hp, qb, pv)

        def proj(b, tq, fos=range(NF)):
            """y^T partial for batch b, 512-token block tq."""
            for fo in fos:
                yp = mm_ps.tile([P, CH], FP, tag="mm")
                for kc in range(2):
                    nc.tensor.matmul(
                        yp[:], wo_t[:, kc, fo * P:(fo + 1) * P],
                        ot[b][kc][:, tq * CH:(tq + 1) * CH],
                        start=(kc == 0), stop=(kc == 1))
                y_sb = stage.tile([P, CH], FP, tag="ysb")
                nc.vector.tensor_copy(y_sb[:], yp[:])
                nc.sync.dma_start(
                    yT[fo * P:(fo + 1) * P,
                       b * S + tq * CH:b * S + (tq + 1) * CH], y_sb[:])

        # ---- emission order ----
        # Prefetch all of batch 0's x, then the minimal lead-in (k/q/v of
        # chunk 0); later chunks feed into block (0,0,0) just in time.
        ldx(0, preamble=True)
        qkv_chunk(0, 0, "k")
        qkv_chunk(0, 0, "q")
        qkv_chunk(0, 0, "v")

        pending = None
        hooks0 = {}
        for c in range(1, NJ):       # k chunk c before round 4c; v likewise
            hooks0[4 * c - 3] = (lambda c=c: qkv_chunk(0, c, "k"))
            hooks0[4 * c - 1] = (lambda c=c: qkv_chunk(0, c, "v"))
        hooks0[13] = lambda: qk_chain(0, 1, "q", 0)
        hooks0[15] = lambda: qk_chain(0, 1, "q", 1)
        pending = attn_block(0, 0, 0, pending, hooks0)

        # attn(b0): remaining b0 q chains + batch-1 chunks 0-1 drip-fed.
        # ldx(1, j) must be emitted after the last b0 reader of xt[j].
        work = [lambda: qk_chain(0, 2, "q", 0),
                lambda: qk_chain(0, 2, "q", 1),
                lambda: qk_chain(0, 3, "q", 0),
                lambda: (qk_chain(0, 3, "q", 1), ldx(1))]
        for c in range(2):
            work += [lambda c=c: qk_chain(1, c, "k", 0),
                     lambda c=c: qk_chain(1, c, "k", 1),
                     lambda c=c: v_pair(1, c, 0),
                     lambda c=c: v_pair(1, c, 2)]
        work += [lambda: v_pair(1, 2, 0), lambda: v_pair(1, 2, 2)]
        work[6:6] = [lambda: qk_chain(1, 0, "q", 0),
                     lambda: qk_chain(1, 0, "q", 1)]
        wi = 0
        for blk, (hp, qb) in enumerate(
                [(hp, qb) for hp in range(2) for qb in range(NQB)]):
            if blk == 0:
                continue
            hooks = {}
            for r in (2, 7, 12):
                if wi < len(work):
                    hooks[r] = work[wi]
                    wi += 1
            pending = attn_block(0, hp, qb, pending, hooks)
        while wi < len(work):
            work[wi]()
            wi += 1

        # attn(b1) qb-outer; b1 k/v chunks 2-3 feed progressively into the
        # first two blocks; q chunks and both proj batches via hooks
        for qb in range(NQB):
            if qb == 0:
                hooks = {1: lambda: qk_chain(1, 2, "k", 0),
                         3: lambda: qk_chain(1, 2, "k", 1),
                         7: lambda: qk_chain(1, 3, "k", 0),
                         9: lambda: qk_chain(1, 3, "k", 1),
                         11: lambda: v_pair(1, 3, 0),
                         13: lambda: v_pair(1, 3, 2)}
            else:
                hooks = {5: (lambda qb=qb: proj(1, qb - 1, range(0, 4))),
                         10: (lambda qb=qb: proj(1, qb - 1, range(4, NF)))}
            pending = attn_block(1, 0, qb, pending, hooks)
            hooks = {5: (lambda qb=qb: proj(0, qb, range(0, 4))),
                     10: (lambda qb=qb: proj(0, qb, range(4, NF)))}
            if qb < NQB - 1:
                hooks[1] = (lambda qb=qb: qk_chain(1, qb + 1, "q", 0))
                hooks[3] = (lambda qb=qb: qk_chain(1, qb + 1, "q", 1))
            pending = attn_block(1, 1, qb, pending, hooks)
        pending()
        proj(1, NQB - 1)

    nc.compile()
    return nc


def build():
    if "nc" not in _cache:
        _cache["nc"] = _build()
    return _cache["nc"]


def make_in_maps(x, Wqkv, bqkv, Wo):
    import ml_dtypes
    mmdt = ml_dtypes.bfloat16
    x = np.ascontiguousarray(np.asarray(x, np.float32))
    Wqkv = np.asarray(Wqkv, np.float32)
    bqkv = np.asarray(bqkv, np.float32)
    Wo = np.asarray(Wo, np.float32)
    in_maps = []
    for c in range(NCORES):
        g, t = divmod(c, TP)
        xTc = np.ascontiguousarray(
            x[g * BL:(g + 1) * BL].reshape(TOK, D).T.astype(mmdt))
        wc = np.ascontiguousarray(np.concatenate(
            [Wqkv[:, i * D + t * HD:i * D + (t + 1) * HD] for i in range(3)],
            axis=1).astype(mmdt))
        bqkc = np.ascontiguousarray(np.concatenate(
            [bqkv[t * HD:(t + 1) * HD],
             bqkv[D + t * HD:D + (t + 1) * HD]]).reshape(2 * HD, 1))
        woc = np.ascontiguousarray(Wo[t * HD:(t + 1) * HD, :].astype(mmdt))
        in_maps.append({"xT": xTc, "w": wc, "bqk": bqkc, "wo": woc})
    return in_maps


LAST_EXEC_NS = None


def kernel(x, Wqkv, bqkv, Wo, bo):
    global LAST_EXEC_NS
    from concourse import bass_utils

    nc = build()
    in_maps = make_in_maps(x, Wqkv, bqkv, Wo)
    res = bass_utils.run_bass_kernel_spmd(
        nc, in_maps, core_ids=list(range(NCORES)))
    LAST_EXEC_NS = res.exec_time_ns
    outs = [r["yT"] for r in res.results]

    Wo = np.asarray(Wo, np.float32)
    bo = np.asarray(bo, np.float32)
    bqkv = np.asarray(bqkv, np.float32)
    hb = bo + np.asarray(bqkv[2 * D:3 * D], np.float32) @ Wo

    halves = []
    for g in range(DP):
        acc = outs[g * TP].astype(np.float32)
        for t in range(1, TP):
            acc = acc + outs[g * TP + t]
        halves.append(acc.T)            # [TOK, D]
    y = np.concatenate(halves, axis=0) + hb[None, :]
    return np.ascontiguousarray(y.reshape(B, S, D).astype(np.float32))


# revision 41
# speedup vs baseline: 1.1075x; 1.1012x over previous
"""Multi-head attention forward, tensor-parallel over 8 TRN2 NeuronCores.

Problem: x[4,2048,1024], Wqkv[1024,3072], bqkv[3072], Wo[1024,1024], bo[1024]
  qkv = x @ Wqkv + bqkv ; 16 heads, d_head 64 ; softmax(QK^T/8) V ; out proj.

Sharding: DP=2 over batch (2 batches/core) x TP=4 over heads (4 heads/core).
Each core computes a partial y^T (its heads' contribution, transposed); the
host sums partials within each batch group, adds biases, and transposes.

Device dataflow (fully SBUF-resident, head-pair row tiling):
  qT,kT = (W_{q,k}^T x^T + b)   [128, 2048] per (batch, head-pair):
                                partitions 0-63 = even head, 64-127 = odd head
  v     = x W_v                 [128 tok, 256] strips -> vtmp -> vo per head
  S^T   = K Q^T per (b,hp,qb,strip): TWO heads concurrently as 64-row PE
          tiles T0/T8 (K=64, operands live on partition halves) -> one
          2-bank PSUM tile [128, 2, 512]
  P^T   = exp(S^T/8)            one ACT call, N=1024, bf16 out
  O^T|s = [V|1]^T P^T           per head, [65, 512] PSUM accumulated over 16
                                strips; row 64 = rowsum
  norm  : reciprocal_approx_fast on rowsums (bug workarounds: 2D base-0 AP
          only), bf16 e65 broadcast matmul, DVE mul -> ot tiles [128, 2048]
          (odd head shifted via SBUF->SBUF DMA)
  y^T  += Wo_part^T O_n^T       [1024, 2048] partial per batch, summed on host

Scheduling: each block's normalization is software-pipelined into round 2 of
the NEXT block (never stalls the PE queue on the DVE reciprocal); projection
chains for the other batch and the output projection are drip-fed into
attention blocks through per-round hooks so the PE never idles long enough
to re-throttle (HAM) and the scalar engine stays saturated.
"""

import sys

if "/opt/trn_rl_repo" not in sys.path:
    sys.path.insert(0, "/opt/trn_rl_repo")

import numpy as np

B, S, D = 4, 2048, 1024
H, DH = 16, 64
NCORES = 8
DP, TP = 2, 4
BL = B // DP            # 2 local batches
TOK = BL * S            # 4096 local tokens
HL = H // TP            # 4 local heads
HD = HL * DH            # 256 local head dims
P = 128
NF = D // P             # 8 contraction chunks
CH = 512                # token chunk for projections
NJ = S // CH            # 4 chunks per batch
NKS = S // P            # 16 k-strips per (batch, head)
QB = 512                # q block (one PSUM bank)
NQB = S // QB           # 4 q blocks per head

_cache = {}


def _build():
    import concourse.bass as bass
    import concourse.tile as tile
    from concourse import bacc, mybir
    from contextlib import ExitStack

    FP = mybir.dt.float32
    FR = mybir.dt.bfloat16
    F32R = mybir.dt.float32r
    AF = mybir.ActivationFunctionType

    nc = bacc.Bacc("TRN2", target_bir_lowering=False, debug=False,
                   num_devices=NCORES)

    xT = nc.dram_tensor("xT", [D, TOK], FR, kind="ExternalInput").ap()
    w = nc.dram_tensor("w", [D, 3 * HD], FR, kind="ExternalInput").ap()
    bqk = nc.dram_tensor("bqk", [2 * HD, 1], FP, kind="ExternalInput").ap()
    wo = nc.dram_tensor("wo", [HD, D], FR, kind="ExternalInput").ap()
    yT = nc.dram_tensor("yT", [D, TOK], FP, kind="ExternalOutput").ap()

    with tile.TileContext(nc) as tc, ExitStack() as ctx:
        konst = ctx.enter_context(tc.tile_pool(name="konst", bufs=1))
        xt_p = ctx.enter_context(tc.tile_pool(name="xt", bufs=1))
        qp_p = ctx.enter_context(tc.tile_pool(name="qp", bufs=1))
        kp_p = ctx.enter_context(tc.tile_pool(name="kp", bufs=1))
        vt_p = ctx.enter_context(tc.tile_pool(name="vt", bufs=1))
        vo_p = ctx.enter_context(tc.tile_pool(name="vo", bufs=1))
        ot_p = ctx.enter_context(tc.tile_pool(name="ot", bufs=1))
        pt_p = ctx.enter_context(tc.tile_pool(name="pt", bufs=12))
        stage = ctx.enter_context(tc.tile_pool(name="stage", bufs=4))
        s_ps = ctx.enter_context(
            tc.tile_pool(name="sps", bufs=2, space="PSUM"))
        pv_ps = ctx.enter_context(
            tc.tile_pool(name="pvps", bufs=1, space="PSUM"))
        mm_ps = ctx.enter_context(
            tc.tile_pool(name="mmps", bufs=2, space="PSUM"))

        # ---- constants resident in SBUF ----
        # (w DMAs are interleaved with the first x-chunk loads below so the
        # first k-projection chain can start as early as possible)
        w_t = konst.tile([P, NF, 3 * HD], FR, tag="w")
        wo_t = konst.tile([P, 2, D], FR, tag="wo")
        bias_t = konst.tile([P, 4], FP, tag="bias")
        nc.sync.dma_start(
            bias_t[:], bqk[:, :].rearrange("(o p) c -> p (o c)", p=P))
        # e65: selects row 64 (the rowsum) in the broadcast matmul (f32r for
        # full PE rate; memset through an fp32 bitcast view)
        e65 = konst.tile([DH + 1, DH], FR, tag="e65")
        nc.gpsimd.memset(e65[:], 0.0)
        nc.gpsimd.memset(e65[DH:DH + 1, :], 1.0)
        # reciprocal staging: row 64 written per norm event; rows 0..63 are
        # constant (multiplied by e65's zeros) but must stay finite
        rcp_t = konst.tile([DH + 1, 2, QB], FR, tag="rcp")
        nc.gpsimd.memset(rcp_t[:], 1.0)
        rcps = konst.tile([DH + 1, 2, QB], FP, tag="rcps")
        rcpi = konst.tile([DH + 1, 2, QB], FP, tag="rcpi")
        nc.gpsimd.memset(rcpi[:], 1.0)
        # fp32 ones row used to fill the vo ones column
        ones16 = konst.tile([P, NKS], FP, tag="ones16")
        nc.gpsimd.memset(ones16[:], 1.0)
        # ACT exp table warm-up (first Exp pays ~2.7us table DMA)
        warm = konst.tile([1, 4], FR, tag="warm")
        nc.scalar.activation(warm[:], bias_t[0:1, :], AF.Exp, scale=0.125)

        # ---- persistent SBUF tensors ----
        # x for one batch (reused batch 1 over batch 0 via WAR deps)
        xt_t = xt_p.tile([P, NF, NJ, CH], FR, tag="xt", name="xt_t")

        # q/k: [128, 2048] per (batch, head-pair); partitions 0-63 even head
        qp = [[qp_p.tile([P, S], FR, tag=f"qp{b}_{hp}", name=f"qp{b}_{hp}")
               for hp in range(2)] for b in range(BL)]
        kp = [[kp_p.tile([P, S], FR, tag=f"kp{b}_{hp}", name=f"kp{b}_{hp}")
               for hp in range(2)] for b in range(BL)]
        # v staging [128 tok, strip, 256 feat] and per-head [V|1] tiles
        vtmp = [vt_p.tile([P, NKS, HD], FR, tag=f"vt{b}", name=f"vt{b}")
                for b in range(BL)]
        vo = [vo_p.tile([P, NKS, HL, DH + 1], FR, tag=f"vo{b}",
                        name=f"vo{b}") for b in range(BL)]
        for b in range(BL):
            for h in range(HL):
                nc.vector.tensor_copy(vo[b][:, :, h, DH], ones16[:])
        # normalized O^T, 2 heads stacked per tile
        ot = [[ot_p.tile([P, S], FR, tag=f"ot{b}_{hp}", name=f"ot{b}_{hp}")
               for hp in range(2)] for b in range(BL)]

        def ldx(b, preamble=False):
            """Queue x DMAs for batch b. The Sync engine issues descriptors
            serially (~0.85us each), so transfers are batched two f-blocks
            per DMA via a rearranged source pattern; chunk 0 comes first
            (interleaved with w in the preamble) so the first k chain is fed
            as early as possible."""
            for g in range(NF // 2):
                nc.sync.dma_start(
                    xt_t[:, 2 * g:2 * g + 2, 0, :],
                    xT[2 * g * P:(2 * g + 2) * P, b * S:b * S + CH]
                    .rearrange("(f p) c -> p f c", p=P))
                if preamble:
                    nc.sync.dma_start(
                        w_t[:, 2 * g:2 * g + 2, :],
                        w[2 * g * P:(2 * g + 2) * P, :]
                        .rearrange("(f p) c -> p f c", p=P))
            for g in range(NF // 2):
                nc.sync.dma_start(
                    xt_t[:, 2 * g:2 * g + 2, 1:NJ, :]
                    .rearrange("p f j c -> p f (j c)"),
                    xT[2 * g * P:(2 * g + 2) * P, b * S + CH:(b + 1) * S]
                    .rearrange("(f p) c -> p f c", p=P))
            if preamble:
                nc.sync.dma_start(
                    wo_t[:, :, :],
                    wo[:, :].rearrange("(kc p) c -> p kc c", p=P))

        def qk_chain(b, j, what, hp):
            """One q/k projection chain (8 MMs) for head pair hp."""
            t0 = j * CH
            base = 0 if what == "q" else HD
            bo_ = 0 if what == "q" else 2
            dst = qp if what == "q" else kp
            ps = mm_ps.tile([P, CH], FP, tag="mm")
            for f in range(NF):
                nc.tensor.matmul(
                    ps[:], w_t[:, f, base + hp * P:base + (hp + 1) * P],
                    xt_t[:, f, j, :],
                    start=(f == 0), stop=(f == NF - 1))
            nc.vector.tensor_scalar_add(
                dst[b][hp][:, t0:t0 + CH], ps[:],
                bias_t[:, bo_ + hp:bo_ + hp + 1])

        def v_pair(b, j, mm0):
            """v projection for two 128-token strips of chunk j."""
            for m in (mm0, mm0 + 1):
                ps = mm_ps.tile([P, CH], FP, tag="mm")
                for f in range(NF):
                    nc.tensor.matmul(
                        ps[:, :HD], xt_t[:, f, j, m * P:(m + 1) * P],
                        w_t[:, f, 2 * HD:3 * HD],
                        start=(f == 0), stop=(f == NF - 1))
                nc.vector.tensor_copy(
                    vtmp[b][:, j * (CH // P) + m, :], ps[:, :HD])
            c0 = j * (CH // P) + mm0
            nc.sync.dma_start(       # scatter these two strips, all heads
                vo[b][:, c0:c0 + 2, :, :DH],
                vtmp[b][:, c0:c0 + 2, :].rearrange(
                    "p s (h c) -> p s h c", c=DH))

        def qkv_chunk(b, j, what):
            """Full chunk projection (both head pairs / all strips)."""
            for i in range(2):
                if what == "v":
                    v_pair(b, j, 2 * i)
                else:
                    qk_chain(b, j, what, i)

        def norm_dve(pv):
            """Reciprocal chain for a finished pv — emitted right at block
            end so it runs on DVE while the next block's rounds start."""
            nc.vector.tensor_copy(rcpi[DH:DH + 1, :, :], pv[DH:DH + 1, :, :])
            # approx reciprocal mis-executes on single-partition base-64 APs;
            # run it over rows 0..64 (rows 0..63 hold a harmless memset 1.0)
            nc.vector.reciprocal_approx_fast(
                rcps[:, :, :].rearrange("p a b -> p (a b)"),
                rcpi[:, :, :].rearrange("p a b -> p (a b)"))
            nc.vector.tensor_copy(rcp_t[DH:DH + 1, :, :],
                                  rcps[DH:DH + 1, :, :])

        def norm(b, hp, qb, pv):
            """Broadcast + scale part of normalization; emitted mid-way
            through the NEXT block so the broadcast matmuls never stall the
            PE queue waiting on the DVE reciprocal."""
            q0 = qb * QB
            for u in range(2):
                bc = mm_ps.tile([P, QB], FP, tag="mm")
                nc.tensor.matmul(bc[:DH, :], e65[:], rcp_t[:, u, :],
                                 start=True, stop=True)
                rb = stage.tile([DH, QB], FP, tag="rb")
                nc.vector.tensor_copy(rb[:], bc[:DH, :])
                if u == 0:
                    nc.vector.tensor_mul(
                        ot[b][hp][0:DH, q0:q0 + QB], pv[0:DH, u, :], rb[:])
                else:
                    on = stage.tile([DH, QB], FR, tag="on")
                    nc.vector.tensor_mul(on[:], pv[0:DH, u, :], rb[:])
                    nc.sync.dma_start(
                        ot[b][hp][DH:2 * DH, q0:q0 + QB], on[:])

        def attn_block(b, hp, qb, pending, hooks=None):
            """S/exp/PV for one (batch, head pair, 512-query block).

            `pending` is the previous block's deferred norm closure (emitted
            after round 2); returns this block's norm closure."""
            q0 = qb * QB
            pv = pv_ps.tile([DH + 1, 2, QB], FP, tag="pv", name="pv")
            for r in range(NKS):
                if hooks and r in hooks:
                    hooks[r]()
                sp = s_ps.tile([P, 2, QB], FP, tag="s")
                for u in range(2):   # u=0: even head (T0), u=1: odd (T8)
                    lo = u * DH
                    nc.tensor.matmul(
                        sp[:, u, :],
                        kp[b][hp][lo:lo + DH, r * P:(r + 1) * P],
                        qp[b][hp][lo:lo + DH, q0:q0 + QB],
                        start=True, stop=True)
                pt = pt_p.tile([P, 2, QB], FR, tag="pt")
                nc.scalar.activation(pt[:], sp[:], AF.Exp, scale=0.125)
                for u in range(2):
                    nc.tensor.matmul(
                        pv[:, u, :], vo[b][:, r, 2 * hp + u, :],
                        pt[:, u, :],
                        start=(r == 0), stop=(r == NKS - 1))
                if r == 2 and pending is not None:
                    pending()
            norm_dve(pv)
            return lambda: norm(b, hp, qb, pv)

        def proj(b, tq, fos=range(NF)):
            """y^T partial for batch b, 512-token block tq; two feature
            blocks share one output DMA (Sync-engine descriptor issue is
            the scarce resource)."""
            fos = list(fos)
            for g in range(0, len(fos), 2):
                y2 = stage.tile([P, 2, CH], FP, tag="ysb2")
                for i in range(2):
                    fo = fos[g + i]
                    yp = mm_ps.tile([P, CH], FP, tag="mm")
                    for kc in range(2):
                        nc.tensor.matmul(
                            yp[:], wo_t[:, kc, fo * P:(fo + 1) * P],
                            ot[b][kc][:, tq * CH:(tq + 1) * CH],
                            start=(kc == 0), stop=(kc == 1))
                    nc.vector.tensor_copy(y2[:, i, :], yp[:])
                fo0 = fos[g]
                nc.sync.dma_start(
                    yT[fo0 * P:(fo0 + 2) * P,
                       b * S + tq * CH:b * S + (tq + 1) * CH]
                    .rearrange("(f p) c -> p f c", p=P), y2[:])

        # ---- emission order ----
        # Prefetch all of batch 0's x, then the minimal lead-in (k/q/v of
        # chunk 0); later chunks feed into block (0,0,0) just in time.
        ldx(0, preamble=True)
        qkv_chunk(0, 0, "k")
        qkv_chunk(0, 0, "q")
        qkv_chunk(0, 0, "v")

        pending = None
        hooks0 = {}
        for c in range(1, NJ):       # k chunk c before round 4c; v likewise
            hooks0[4 * c - 3] = (lambda c=c: qkv_chunk(0, c, "k"))
            hooks0[4 * c - 1] = (lambda c=c: qkv_chunk(0, c, "v"))
        hooks0[13] = lambda: qk_chain(0, 1, "q", 0)
        hooks0[15] = lambda: qk_chain(0, 1, "q", 1)
        pending = attn_block(0, 0, 0, pending, hooks0)

        # attn(b0): remaining b0 q chains + batch-1 chunks 0-1 drip-fed.
        # ldx(1, j) must be emitted after the last b0 reader of xt[j].
        work = [lambda: qk_chain(0, 2, "q", 0),
                lambda: qk_chain(0, 2, "q", 1),
                lambda: qk_chain(0, 3, "q", 0),
                lambda: (qk_chain(0, 3, "q", 1), ldx(1))]
        for c in range(2):
            work += [lambda c=c: qk_chain(1, c, "k", 0),
                     lambda c=c: qk_chain(1, c, "k", 1),
                     lambda c=c: v_pair(1, c, 0),
                     lambda c=c: v_pair(1, c, 2)]
        work += [lambda: v_pair(1, 2, 0), lambda: v_pair(1, 2, 2)]
        work[6:6] = [lambda: qk_chain(1, 0, "q", 0),
                     lambda: qk_chain(1, 0, "q", 1)]
        wi = 0
        for blk, (hp, qb) in enumerate(
                [(hp, qb) for hp in range(2) for qb in range(NQB)]):
            if blk == 0:
                continue
            hooks = {}
            for r in (2, 7, 12):
                if wi < len(work):
                    hooks[r] = work[wi]
                    wi += 1
            pending = attn_block(0, hp, qb, pending, hooks)
        while wi < len(work):
            work[wi]()
            wi += 1

        # attn(b1) qb-outer; b1 k/v chunks 2-3 feed progressively into the
        # first two blocks; q chunks and both proj batches via hooks
        for qb in range(NQB):
            if qb == 0:
                hooks = {1: lambda: qk_chain(1, 2, "k", 0),
                         3: lambda: qk_chain(1, 2, "k", 1),
                         7: lambda: qk_chain(1, 3, "k", 0),
                         9: lambda: qk_chain(1, 3, "k", 1),
                         11: lambda: v_pair(1, 3, 0),
                         13: lambda: v_pair(1, 3, 2)}
            else:
                hooks = {5: (lambda qb=qb: proj(1, qb - 1, range(0, 4))),
                         10: (lambda qb=qb: proj(1, qb - 1, range(4, NF)))}
            pending = attn_block(1, 0, qb, pending, hooks)
            hooks = {5: (lambda qb=qb: proj(0, qb, range(0, 4))),
                     10: (lambda qb=qb: proj(0, qb, range(4, NF)))}
            if qb < NQB - 1:
                hooks[1] = (lambda qb=qb: qk_chain(1, qb + 1, "q", 0))
                hooks[3] = (lambda qb=qb: qk_chain(1, qb + 1, "q", 1))
            pending = attn_block(1, 1, qb, pending, hooks)
        pending()
        proj(1, NQB - 1)

    nc.compile()
    return nc


def build():
    if "nc" not in _cache:
        _cache["nc"] = _build()
    return _cache["nc"]


def make_in_maps(x, Wqkv, bqkv, Wo):
    import ml_dtypes
    mmdt = ml_dtypes.bfloat16
    x = np.ascontiguousarray(np.asarray(x, np.float32))
    Wqkv = np.asarray(Wqkv, np.float32)
    bqkv = np.asarray(bqkv, np.float32)
    Wo = np.asarray(Wo, np.float32)
    in_maps = []
    for c in range(NCORES):
        g, t = divmod(c, TP)
        xTc = np.ascontiguousarray(
            x[g * BL:(g + 1) * BL].reshape(TOK, D).T.astype(mmdt))
        wc = np.ascontiguousarray(np.concatenate(
            [Wqkv[:, i * D + t * HD:i * D + (t + 1) * HD] for i in range(3)],
            axis=1).astype(mmdt))
        bqkc = np.ascontiguousarray(np.concatenate(
            [bqkv[t * HD:(t + 1) * HD],
             bqkv[D + t * HD:D + (t + 1) * HD]]).reshape(2 * HD, 1))
        woc = np.ascontiguousarray(Wo[t * HD:(t + 1) * HD, :].astype(mmdt))
        in_maps.append({"xT": xTc, "w": wc, "bqk": bqkc, "wo": woc})
    return in_maps


LAST_EXEC_NS = None


def kernel(x, Wqkv, bqkv, Wo, bo):
    global LAST_EXEC_NS
    from concourse import bass_utils

    nc = build()
    in_maps = make_in_maps(x, Wqkv, bqkv, Wo)
    res = bass_utils.run_bass_kernel_spmd(
        nc, in_maps, core_ids=list(range(NCORES)))
    LAST_EXEC_NS = res.exec_time_ns
    outs = [r["yT"] for r in res.results]

    Wo = np.asarray(Wo, np.float32)
    bo = np.asarray(bo, np.float32)
    bqkv = np.asarray(bqkv, np.float32)
    hb = bo + np.asarray(bqkv[2 * D:3 * D], np.float32) @ Wo

    halves = []
    for g in range(DP):
        acc = outs[g * TP].astype(np.float32)
        for t in range(1, TP):
            acc = acc + outs[g * TP + t]
        halves.append(acc.T)            # [TOK, D]
    y = np.concatenate(halves, axis=0) + hb[None, :]
    return np.ascontiguousarray(y.reshape(B, S, D).astype(np.float32))
